# revision 36
# baseline (speedup 1.0000x reference)
"""Trainium2 Bass kernel for ConvMHSA (B=16, C=512, H=W=32, 8 heads).

Data-parallel over batch: each of the 8 NeuronCores processes 2 batches.

Layout strategy per batch (xf = x reshaped to (C=512, N=1024), bf16):
  - Q, K projected into (C, N) layout, quantized to fp8e4 on the bias-add,
    then partition-folded per head to [32, j=2, N] via an SBUF->SBUF DMA
    (channel c lives at (p=c//2, j=c%2) - same bijection for both sides).
  - V projected TRANSPOSED: vT (N_mtile, head, hd+1) bf16 with a ones
    column appended per head -> PV emits softmax denominators Z for free.
  - Scores transposed: S^T[m, n] = sum_c k[c,m] q[c,n] per head, computed
    as fp8 DoubleRow matmuls (0.5 PE cycles/row - half the fp32r cost).
  - exp fused into the PSUM->SBUF copy on ScalarE, output bf16.
  - PV computed in the n-partition orientation: for each head h and
    128-wide n-chunk, psum[n, hd+1] = sum_m e[m, n-chunk]^T [vT_h | 1],
    accumulated over the 8 m-tiles.  Moving dim is only hd+1=65 (bf16,
    1 cyc/row), halving PV PE time vs the hd-partition orientation.
    Z lands in column 64 => normalization is a per-partition scalar:
    reciprocal_approx_fast + one tensor_scalar_mul per (head, chunk).
    No cross-partition broadcast (no DRAM round-trip) needed.
  - Normalized attn^T pairs (two heads side by side, [128 n, 128 c])
    are transposed back to (c, n) with PE transpose (bf16, 128 rows)
    and copied to SBUF for the output projection.
  - Output projection in bf16 with gamma folded into the weights on the
    host; residual x added from the bf16 xf tiles.

Emission order software-pipelines projection, attention and PV at
head-pair granularity so ScalarE (exp, ~133us/core, the floor of this
kernel) never starves and the PE stays packed.
"""

import os
import sys

sys.path.insert(0, "/opt/trn_rl_repo")

import numpy as np

B, C, H, W = 16, 512, 32, 32
HEADS = 8
HD = C // HEADS          # 64
N = H * W                # 1024
NCORES = 8
NB = B // NCORES         # batches per core = 2
KT = C // 128            # 4 contraction tiles of 128
NCH = N // 512           # 2 moving chunks of 512
MT = N // 128            # 8 m-tiles
NPAIR = HEADS // 2       # 4 head-pairs

_cache = {}


def _build_nc():
    import concourse.bass as bass
    import concourse.tile as tile
    import concourse.mybir as mybir
    from concourse import bacc

    F32 = mybir.dt.float32
    F32R = mybir.dt.float32r
    BF16 = mybir.dt.bfloat16
    F8 = mybir.dt.float8e4
    EXP = mybir.ActivationFunctionType.Exp

    nc = bacc.Bacc("TRN2", target_bir_lowering=False, debug=False,
                   num_devices=NCORES)

    xs = nc.dram_tensor("xs", (NB, C, N), BF16, kind="ExternalInput").ap()
    wqkvT = nc.dram_tensor("wqkvT", (C, 3 * C), BF16, kind="ExternalInput").ap()
    bqkv_col = nc.dram_tensor("bqkv_col", (128, 12), F32, kind="ExternalInput").ap()
    bqkv_row = nc.dram_tensor("bqkv_row", (1, 3 * C), F32R, kind="ExternalInput").ap()
    woTg = nc.dram_tensor("woTg", (C, C), BF16, kind="ExternalInput").ap()
    bog_col = nc.dram_tensor("bog_col", (128, KT), F32, kind="ExternalInput").ap()
    identd = nc.dram_tensor("identd", (128, 128), BF16, kind="ExternalInput").ap()
    y = nc.dram_tensor("y", (NB, C, N), F32, kind="ExternalOutput").ap()

    with tile.TileContext(nc) as tc:
        with tc.tile_pool(name="const", bufs=1) as const, \
             tc.tile_pool(name="big", bufs=1) as big, \
             tc.tile_pool(name="dbuf", bufs=2) as dbuf, \
             tc.tile_pool(name="epool", bufs=24) as epool, \
             tc.tile_pool(name="apool", bufs=2) as apool, \
             tc.tile_pool(name="tpool", bufs=12) as tpool, \
             tc.tile_pool(name="small", bufs=4) as small, \
             tc.tile_pool(name="osb", bufs=4) as osbp, \
             tc.tile_pool(name="spool", bufs=2, space="PSUM") as spool, \
             tc.tile_pool(name="mm", bufs=2, space="PSUM") as mm, \
             tc.tile_pool(name="pvp", bufs=2, space="PSUM") as pvp:

            # ---- startup loads on parallel DMA queues: xf (critical path
            # for the first projections) split across SP + Activation
            # queues, weights on the gpsimd queue, small tensors last.
            xf0 = [None] * KT
            wq = [None] * KT
            for kc in range(KT):
                t = dbuf.tile([128, N], BF16, tag=f"xf{kc}", name=f"xf{kc}")
                eng = nc.sync if kc % 2 == 0 else nc.scalar
                eng.dma_start(out=t, in_=xs[0, 128 * kc:128 * (kc + 1), :])
                xf0[kc] = t
            bq_col = const.tile([128, 12], F32, tag="bqcol")
            nc.scalar.dma_start(out=bq_col, in_=bqkv_col)
            for kc in range(KT):
                wq[kc] = const.tile([128, 3 * C], BF16, tag=f"wq{kc}",
                                    name=f"wq{kc}")
                nc.gpsimd.dma_start(
                    out=wq[kc][:, 0:2 * C],
                    in_=wqkvT[128 * kc:128 * (kc + 1), 0:2 * C])
            for kc in range(KT):
                nc.gpsimd.dma_start(
                    out=wq[kc][:, 2 * C:3 * C],
                    in_=wqkvT[128 * kc:128 * (kc + 1), 2 * C:3 * C])
            bv_bc = const.tile([128, C], F32R, tag="bvbc")
            bv_src = bass.AP(tensor=bqkv_row.tensor, offset=2 * C,
                             ap=[[0, 128], [1, C]])
            nc.sync.dma_start(out=bv_bc, in_=bv_src)
            wo = []
            for kc in range(KT):
                t = const.tile([128, C], BF16, tag=f"wo{kc}", name=f"wo{kc}")
                nc.scalar.dma_start(out=t, in_=woTg[128 * kc:128 * (kc + 1), :])
                wo.append(t)
            ident = const.tile([128, 128], BF16, tag="ident")
            nc.sync.dma_start(out=ident, in_=identd)
            bo_col = const.tile([128, KT], F32, tag="bocol")
            nc.sync.dma_start(out=bo_col, in_=bog_col)
            ones_h = const.tile([128, HEADS], BF16, tag="onesh")
            nc.vector.memset(ones_h, 1.0)

            def load_xf(b):
                if b == 0:
                    return xf0
                out = []
                for kc in range(KT):
                    t = dbuf.tile([128, N], BF16, tag=f"xf{kc}",
                                  name=f"xf{kc}")
                    nc.sync.dma_start(out=t,
                                      in_=xs[b, 128 * kc:128 * (kc + 1), :])
                    out.append(t)
                return out

            def proj_qk_alloc(pair, which):
                return big.tile([128, N], F8, tag=f"{which}{pair}",
                                name=f"{which}{pair}")

            def fold_alloc(pair, which):
                # [32, head, j, n]: channel c of a head lives at
                # (p=c//2, j=c%2) - same bijection for q and k.
                return big.tile([32, 2, 2, N], F8, tag=f"{which}f{pair}",
                                name=f"{which}f{pair}")

            def fold_qk(src8, dstf, hh):
                nc.gpsimd.dma_start(out=dstf[:, hh],
                                    in_=src8[64 * hh:64 * (hh + 1), :])

            def proj_qk_chunk(xf, pair, which, nch, dest):
                ot = pair if which == "q" else KT + pair
                ps = mm.tile([128, 512], F32, tag="mm", name="ps")
                for kc in range(KT):
                    nc.tensor.matmul(
                        ps,
                        wq[kc][:, 128 * ot:128 * (ot + 1)],
                        xf[kc][:, 512 * nch:512 * (nch + 1)],
                        start=(kc == 0), stop=(kc == KT - 1))
                nc.vector.tensor_scalar_add(
                    out=dest[:, 512 * nch:512 * (nch + 1)],
                    in0=ps, scalar1=bq_col[:, ot:ot + 1])

            def proj_vT_alloc():
                return [dbuf.tile([128, HEADS, HD + 1], BF16, tag=f"vT{mt}",
                                  name=f"vT{mt}") for mt in range(MT)]

            def proj_vT_mtile(xf, vt, mt):
                ps = mm.tile([128, 512], F32, tag="mm", name="ps")
                for kc in range(KT):
                    nc.tensor.matmul(
                        ps,
                        xf[kc][:, 128 * mt:128 * (mt + 1)],
                        wq[kc][:, 2 * C:3 * C],
                        start=(kc == 0), stop=(kc == KT - 1))
                nc.vector.tensor_add(
                    out=vt[:, :, 0:HD],
                    in0=ps.rearrange("p (a b) -> p a b", a=HEADS),
                    in1=bv_bc.bitcast(F32).rearrange("p (a b) -> p a b",
                                                     a=HEADS))
                nc.vector.tensor_copy(
                    out=vt[:, :, HD:HD + 1],
                    in_=ones_h.rearrange("p (a o) -> p a o", o=1))

            # ---- one PV chain: head hh of a pair, one 128-wide n-chunk.
            # Normalization (recip Z + per-partition scalar mul) happens
            # right after; the mul and the post-transpose copy run on the
            # gpsimd engine to keep DVE clear.
            aT_tiles = {}

            COPY = mybir.ActivationFunctionType.Copy

            def emit_chain(vt_, etiles, pairg, hh, nch, on_act=False):
                bank = pvp.tile([128, 512], F32, tag="pv", name="pvb")
                if hh == 0:
                    aT_tiles[pairg, nch] = tpool.tile(
                        [128, 128], BF16, tag="aT", name="aT")
                aT = aT_tiles[pairg, nch]
                h = 2 * (pairg % NPAIR) + hh
                for j in range(MT):
                    nc.tensor.matmul(
                        bank[:, 0:65],
                        etiles[j][:, 128 * nch:128 * (nch + 1)],
                        vt_[j][:, h, :],
                        start=(j == 0), stop=(j == MT - 1))
                rz = small.tile([128, 1], F32, tag="rz", name="rz")
                nc.vector.reciprocal_approx_fast(out=rz,
                                                 in_=bank[:, 64:65])
                if on_act:
                    nc.scalar.activation(out=aT[:, 64 * hh:64 * hh + 64],
                                         in_=bank[:, 0:64], func=COPY,
                                         scale=rz)
                else:
                    nc.vector.tensor_scalar_mul(
                        out=aT[:, 64 * hh:64 * hh + 64],
                        in0=bank[:, 0:64], scalar1=rz)

            def emit_transpose(attnp, pairg, nch, on_act=False):
                tp = mm.tile([128, 512], F32, tag="mm",
                             name="tp").bitcast(BF16)
                nc.tensor.transpose(tp[:, 0:128], aT_tiles.pop((pairg, nch)),
                                    ident)
                dst = attnp[pairg % NPAIR][:, 128 * nch:128 * (nch + 1)]
                if on_act:
                    nc.scalar.activation(out=dst, in_=tp[:, 0:128], func=COPY)
                else:
                    nc.vector.tensor_copy(out=dst, in_=tp[:, 0:128])

            def stretch(qt, kt, prevP, curP, fillers=(), front=()):
                """QK+exp for this pair.  Steps 0-7 (par 0) also run the
                PREVIOUS pair's head-1 PV chains + transposes; steps 8-15
                (par 1) run THIS pair's head-0 chains (e[0] is complete by
                then).  Chains therefore trail by only half a stretch and
                never pile up at the boundary."""
                ehd = [[], []]
                nf = len(fillers)
                for i in range(16):
                    par, mt = divmod(i, 8)
                    if qt is not None:
                        s = spool.tile([128, N], F32, tag="s", name="s")
                        for c4 in range(4):
                            nc.tensor.matmul(
                                s[:, 256 * c4:256 * (c4 + 1)],
                                kt[:, par, :, 128 * mt:128 * (mt + 1)],
                                qt[:, par, :, 256 * c4:256 * (c4 + 1)],
                                start=True, stop=True,
                                perf_mode=mybir.MatmulPerfMode.DoubleRow)
                        e = epool.tile([128, N], BF16, tag="e", name="e")
                        nc.scalar.activation(out=e, in_=s, func=EXP,
                                             scale=0.125)
                        ehd[par].append(e)
                    if par == 0 and prevP is not None:
                        vtp, ehdp, attnp, pairgp = prevP
                        emit_chain(vtp, ehdp[1], pairgp, 1, mt)
                        emit_transpose(attnp, pairgp, mt)
                    if par == 1 and curP is not None:
                        vtc, pairgc = curP
                        emit_chain(vtc, ehd[0], pairgc, 0, mt)
                    if i < len(front):
                        front[i]()
                    for j in range(nf * i // 16, nf * (i + 1) // 16):
                        fillers[j]()
                return ehd

            def out_proj_chunk(xf, attn, b, ot, nch):
                ps = mm.tile([128, 512], F32, tag="mm", name="ps")
                for kc in range(KT):
                    nc.tensor.matmul(
                        ps,
                        wo[kc][:, 128 * ot:128 * (ot + 1)],
                        attn[kc][:, 512 * nch:512 * (nch + 1)],
                        start=(kc == 0), stop=(kc == KT - 1))
                osb = osbp.tile([128, 512], F32, tag="osb")
                nc.vector.scalar_tensor_tensor(
                    out=osb, in0=ps, scalar=bo_col[:, ot:ot + 1],
                    in1=xf[ot][:, 512 * nch:512 * (nch + 1)],
                    op0=mybir.AluOpType.add, op1=mybir.AluOpType.add)
                nc.sync.dma_start(
                    out=y[b, 128 * ot:128 * (ot + 1),
                          512 * nch:512 * (nch + 1)],
                    in_=osb)

            # ---- software-pipelined emission ----
            attn_all, xf_all, vT_all = {}, {0: xf0}, {}
            q_all, k_all, e_all = {}, {}, {}
            qf_all, kf_all = {}, {}
            for b in range(NB):
                attn_all[b] = [apool.tile([128, N], BF16, tag=f"attn{t}",
                                          name=f"attn{t}") for t in range(KT)]
                vT_all[b] = None
                for pr in range(NPAIR):
                    q_all[b, pr] = proj_qk_alloc(pr, "q")
                    k_all[b, pr] = proj_qk_alloc(pr, "k")
                    qf_all[b, pr] = fold_alloc(pr, "q")
                    kf_all[b, pr] = fold_alloc(pr, "k")

            def mkjob(fn, *a):
                return lambda: fn(*a)

            def qk_jobs(b, pr):
                xf = xf_all[b]
                jobs = []
                for w in ("q", "k"):
                    src8 = (q_all if w == "q" else k_all)[b, pr]
                    dstf = (qf_all if w == "q" else kf_all)[b, pr]
                    jobs += [mkjob(proj_qk_chunk, xf, pr, w, nch, src8)
                             for nch in range(NCH)]
                    jobs += [mkjob(fold_qk, src8, dstf, hh)
                             for hh in range(2)]
                return jobs

            def vt_jobs(b):
                vT_all[b] = proj_vT_alloc()
                xf = xf_all[b]
                return [mkjob(proj_vT_mtile, xf, vT_all[b][mt], mt)
                        for mt in range(MT)]

            def op_jobs(b):
                return [mkjob(out_proj_chunk, xf_all[b], attn_all[b], b,
                              ot, nch)
                        for nch in range(NCH) for ot in range(KT)]

            def prev_of(b, p):
                if (b, p) == (0, 0):
                    return None
                pb, pp = (b, p - 1) if p > 0 else (b - 1, NPAIR - 1)
                return (vT_all[pb], e_all[pb, pp], attn_all[pb],
                        pb * NPAIR + pp)

            # startup: q/k for (0, 0) emitted plainly
            for j in qk_jobs(0, 0):
                j()
            for b in range(NB):
                for pr in range(NPAIR):
                    fillers = []
                    front = []
                    if pr + 1 < NPAIR:
                        fillers += qk_jobs(b, pr + 1)
                    elif b + 1 < NB:
                        fillers += qk_jobs(b + 1, 0)
                    if pr == 0 and b == 0:
                        front = vt_jobs(0)
                    if b + 1 < NB and pr == 1:
                        xf_all[b + 1] = load_xf(b + 1)
                        vt_next = vt_jobs(b + 1)
                        fillers += vt_next[:4]
                    if b + 1 < NB and pr == 2:
                        fillers += vt_next[4:]
                    if b > 0 and pr == 0:
                        fillers += op_jobs(b - 1)[:4]
                    if b > 0 and pr == 1:
                        fillers += op_jobs(b - 1)[4:]

                    e_all[b, pr] = stretch(qf_all[b, pr], kf_all[b, pr],
                                           prev_of(b, pr),
                                           (vT_all[b], b * NPAIR + pr),
                                           fillers, front)
            # drain: last pair's head-1 chains + transposes interleaved
            # with the out-proj finals (kc3 + SBUF partial + store).
            vtp, ehdp, attnp, pairgp = prev_of(NB, 0)
            ops = op_jobs(NB - 1)
            for nch in range(MT):
                emit_chain(vtp, ehdp[1], pairgp, 1, nch)
                emit_transpose(attnp, pairgp, nch)
                if nch >= 3:
                    ops[nch - 3]()
            for j in ops[5:]:
                j()

    nc.compile()
    return nc


def kernel(x, qkv_w, qkv_b, out_w, out_b, gamma):
    import ml_dtypes
    from concourse.bass_utils import run_bass_kernel_spmd

    x = np.asarray(x, dtype=np.float32)
    qkv_w = np.asarray(qkv_w, dtype=np.float32)
    qkv_b = np.asarray(qkv_b, dtype=np.float32)
    out_w = np.asarray(out_w, dtype=np.float32)
    out_b = np.asarray(out_b, dtype=np.float32)
    gamma = np.asarray(gamma, dtype=np.float32)

    if "nc" not in _cache:
        _cache["nc"] = _build_nc()
    nc = _cache["nc"]

    xf = np.ascontiguousarray(
        x.reshape(B, C, N).astype(ml_dtypes.bfloat16))
    wqkvT = np.ascontiguousarray(
        qkv_w.T.astype(ml_dtypes.bfloat16))                  # (C, 3C) bf16
    bq_col = np.ascontiguousarray(qkv_b.reshape(12, 128).T)  # (128, 12)
    bq_row = np.ascontiguousarray(qkv_b.reshape(1, 3 * C))
    g = gamma.reshape(-1)[0]
    woTg = np.ascontiguousarray(
        (g * out_w).T.astype(ml_dtypes.bfloat16))            # (C, C) bf16
    bog_col = np.ascontiguousarray((g * out_b).reshape(KT, 128).T)
    ident = np.eye(128, dtype=ml_dtypes.bfloat16)

    in_maps = []
    for c in range(NCORES):
        in_maps.append({
            "xs": np.ascontiguousarray(xf[NB * c:NB * (c + 1)]),
            "wqkvT": wqkvT,
            "bqkv_col": bq_col,
            "bqkv_row": bq_row,
            "woTg": woTg,
            "bog_col": bog_col,
            "identd": ident,
        })

    trace = bool(int(os.environ.get("KERNEL_TRACE", "0")))
    try:
        res = run_bass_kernel_spmd(nc, in_maps, core_ids=list(range(NCORES)),
                                   trace=trace)
    except ModuleNotFoundError:
        # NTFF profiling hooks unavailable under this axon client
        res = run_bass_kernel_spmd(nc, in_maps, core_ids=list(range(NCORES)),
                                   trace=False)
    _cache["last_result"] = res

    out = np.concatenate([res.results[c]["y"] for c in range(NCORES)], axis=0)
    return out.reshape(B, C, H, W)


# revision 38
# speedup vs baseline: 1.0079x; 1.0079x over previous
"""Trainium2 Bass kernel for ConvMHSA (B=16, C=512, H=W=32, 8 heads).

Data-parallel over batch: each of the 8 NeuronCores processes 2 batches.

Layout strategy per batch (xf = x reshaped to (C=512, N=1024), bf16):
  - Q, K projected into (C, N) layout, quantized to fp8e4 on the bias-add,
    then partition-folded per head to [32, j=2, N] via an SBUF->SBUF DMA
    (channel c lives at (p=c//2, j=c%2) - same bijection for both sides).
  - V projected TRANSPOSED: vT (N_mtile, head, hd+1) bf16 with a ones
    column appended per head -> PV emits softmax denominators Z for free.
  - Scores transposed: S^T[m, n] = sum_c k[c,m] q[c,n] per head, computed
    as fp8 DoubleRow matmuls (0.5 PE cycles/row - half the fp32r cost).
  - exp fused into the PSUM->SBUF copy on ScalarE, output bf16.
  - PV computed in the n-partition orientation: for each head h and
    128-wide n-chunk, psum[n, hd+1] = sum_m e[m, n-chunk]^T [vT_h | 1],
    accumulated over the 8 m-tiles.  Moving dim is only hd+1=65 (bf16,
    1 cyc/row), halving PV PE time vs the hd-partition orientation.
    Z lands in column 64 => normalization is a per-partition scalar:
    reciprocal_approx_fast + one tensor_scalar_mul per (head, chunk).
    No cross-partition broadcast (no DRAM round-trip) needed.
  - Normalized attn^T pairs (two heads side by side, [128 n, 128 c])
    are transposed back to (c, n) with PE transpose (bf16, 128 rows)
    and copied to SBUF for the output projection.
  - Output projection in bf16 with gamma folded into the weights on the
    host; residual x added from the bf16 xf tiles.

Emission order software-pipelines projection, attention and PV at
head-pair granularity so ScalarE (exp, ~133us/core, the floor of this
kernel) never starves and the PE stays packed.
"""

import os
import sys

sys.path.insert(0, "/opt/trn_rl_repo")

import numpy as np

B, C, H, W = 16, 512, 32, 32
HEADS = 8
HD = C // HEADS          # 64
N = H * W                # 1024
NCORES = 8
NB = B // NCORES         # batches per core = 2
KT = C // 128            # 4 contraction tiles of 128
NCH = N // 512           # 2 moving chunks of 512
MT = N // 128            # 8 m-tiles
NPAIR = HEADS // 2       # 4 head-pairs

_cache = {}


def _build_nc():
    import concourse.bass as bass
    import concourse.tile as tile
    import concourse.mybir as mybir
    from concourse import bacc

    F32 = mybir.dt.float32
    F32R = mybir.dt.float32r
    BF16 = mybir.dt.bfloat16
    F8 = mybir.dt.float8e4
    EXP = mybir.ActivationFunctionType.Exp

    nc = bacc.Bacc("TRN2", target_bir_lowering=False, debug=False,
                   num_devices=NCORES)

    xs = nc.dram_tensor("xs", (NB, C, N), BF16, kind="ExternalInput").ap()
    wqkvT = nc.dram_tensor("wqkvT", (C, 3 * C), BF16, kind="ExternalInput").ap()
    bqkv_col = nc.dram_tensor("bqkv_col", (128, 12), F32, kind="ExternalInput").ap()
    bqkv_row = nc.dram_tensor("bqkv_row", (1, 3 * C), F32R, kind="ExternalInput").ap()
    woTg = nc.dram_tensor("woTg", (C, C), BF16, kind="ExternalInput").ap()
    bog_col = nc.dram_tensor("bog_col", (128, KT), F32, kind="ExternalInput").ap()
    identd = nc.dram_tensor("identd", (128, 128), BF16, kind="ExternalInput").ap()
    y = nc.dram_tensor("y", (NB, C, N), F32, kind="ExternalOutput").ap()

    with tile.TileContext(nc) as tc:
        with tc.tile_pool(name="const", bufs=1) as const, \
             tc.tile_pool(name="big", bufs=1) as big, \
             tc.tile_pool(name="dbuf", bufs=2) as dbuf, \
             tc.tile_pool(name="epool", bufs=24) as epool, \
             tc.tile_pool(name="apool", bufs=2) as apool, \
             tc.tile_pool(name="tpool", bufs=12) as tpool, \
             tc.tile_pool(name="small", bufs=4) as small, \
             tc.tile_pool(name="osb", bufs=4) as osbp, \
             tc.tile_pool(name="spool", bufs=2, space="PSUM") as spool, \
             tc.tile_pool(name="mm", bufs=2, space="PSUM") as mm, \
             tc.tile_pool(name="pvp", bufs=2, space="PSUM") as pvp:

            # ---- startup loads on parallel DMA queues: xf (critical path
            # for the first projections) split across SP + Activation
            # queues, weights on the gpsimd queue, small tensors last.
            xf0 = [None] * KT
            wq = [None] * KT
            for kc in range(KT):
                t = dbuf.tile([128, N], BF16, tag=f"xf{kc}", name=f"xf{kc}")
                eng = nc.sync if kc % 2 == 0 else nc.scalar
                eng.dma_start(out=t, in_=xs[0, 128 * kc:128 * (kc + 1), :])
                xf0[kc] = t
            bq_col = const.tile([128, 12], F32, tag="bqcol")
            nc.scalar.dma_start(out=bq_col, in_=bqkv_col)
            for kc in range(KT):
                wq[kc] = const.tile([128, 3 * C], BF16, tag=f"wq{kc}",
                                    name=f"wq{kc}")
                nc.gpsimd.dma_start(
                    out=wq[kc][:, 0:2 * C],
                    in_=wqkvT[128 * kc:128 * (kc + 1), 0:2 * C])
            for kc in range(KT):
                nc.gpsimd.dma_start(
                    out=wq[kc][:, 2 * C:3 * C],
                    in_=wqkvT[128 * kc:128 * (kc + 1), 2 * C:3 * C])
            bv_bc = const.tile([128, C], F32R, tag="bvbc")
            bv_src = bass.AP(tensor=bqkv_row.tensor, offset=2 * C,
                             ap=[[0, 128], [1, C]])
            nc.sync.dma_start(out=bv_bc, in_=bv_src)
            wo = []
            for kc in range(KT):
                t = const.tile([128, C], BF16, tag=f"wo{kc}", name=f"wo{kc}")
                nc.scalar.dma_start(out=t, in_=woTg[128 * kc:128 * (kc + 1), :])
                wo.append(t)
            ident = const.tile([128, 128], BF16, tag="ident")
            nc.sync.dma_start(out=ident, in_=identd)
            bo_col = const.tile([128, KT], F32, tag="bocol")
            nc.sync.dma_start(out=bo_col, in_=bog_col)
            ones_h = const.tile([128, HEADS], BF16, tag="onesh")
            nc.vector.memset(ones_h, 1.0)

            def load_xf(b):
                if b == 0:
                    return xf0
                out = []
                for kc in range(KT):
                    t = dbuf.tile([128, N], BF16, tag=f"xf{kc}",
                                  name=f"xf{kc}")
                    nc.sync.dma_start(out=t,
                                      in_=xs[b, 128 * kc:128 * (kc + 1), :])
                    out.append(t)
                return out

            def proj_qk_alloc(pair, which):
                return big.tile([128, N], F8, tag=f"{which}{pair}",
                                name=f"{which}{pair}")

            def fold_alloc(pair, which):
                # [32, head, j, n]: channel c of a head lives at
                # (p=c//2, j=c%2) - same bijection for q and k.
                return big.tile([32, 2, 2, N], F8, tag=f"{which}f{pair}",
                                name=f"{which}f{pair}")

            def fold_qk(src8, dstf, hh):
                nc.gpsimd.dma_start(out=dstf[:, hh],
                                    in_=src8[64 * hh:64 * (hh + 1), :])

            def proj_qk_chunk(xf, pair, which, nch, dest):
                ot = pair if which == "q" else KT + pair
                ps = mm.tile([128, 512], F32, tag="mm", name="ps")
                for kc in range(KT):
                    nc.tensor.matmul(
                        ps,
                        wq[kc][:, 128 * ot:128 * (ot + 1)],
                        xf[kc][:, 512 * nch:512 * (nch + 1)],
                        start=(kc == 0), stop=(kc == KT - 1))
                nc.vector.tensor_scalar_add(
                    out=dest[:, 512 * nch:512 * (nch + 1)],
                    in0=ps, scalar1=bq_col[:, ot:ot + 1])

            def proj_vT_alloc():
                return [dbuf.tile([128, HEADS, HD + 1], BF16, tag=f"vT{mt}",
                                  name=f"vT{mt}") for mt in range(MT)]

            def proj_vT_mtile(xf, vt, mt):
                ps = mm.tile([128, 512], F32, tag="mm", name="ps")
                for kc in range(KT):
                    nc.tensor.matmul(
                        ps,
                        xf[kc][:, 128 * mt:128 * (mt + 1)],
                        wq[kc][:, 2 * C:3 * C],
                        start=(kc == 0), stop=(kc == KT - 1))
                nc.vector.tensor_add(
                    out=vt[:, :, 0:HD],
                    in0=ps.rearrange("p (a b) -> p a b", a=HEADS),
                    in1=bv_bc.bitcast(F32).rearrange("p (a b) -> p a b",
                                                     a=HEADS))
                nc.vector.tensor_copy(
                    out=vt[:, :, HD:HD + 1],
                    in_=ones_h.rearrange("p (a o) -> p a o", o=1))

            # ---- one PV chain: head hh of a pair, one 128-wide n-chunk.
            # Normalization (recip Z + per-partition scalar mul) happens
            # right after; the mul and the post-transpose copy run on the
            # gpsimd engine to keep DVE clear.
            aT_tiles = {}

            COPY = mybir.ActivationFunctionType.Copy

            def emit_chain(vt_, etiles, pairg, hh, nch, on_act=False):
                bank = pvp.tile([128, 512], F32, tag="pv", name="pvb")
                if hh == 0:
                    aT_tiles[pairg, nch] = tpool.tile(
                        [128, 128], BF16, tag="aT", name="aT")
                aT = aT_tiles[pairg, nch]
                h = 2 * (pairg % NPAIR) + hh
                for j in range(MT):
                    nc.tensor.matmul(
                        bank[:, 0:65],
                        etiles[j][:, 128 * nch:128 * (nch + 1)],
                        vt_[j][:, h, :],
                        start=(j == 0), stop=(j == MT - 1))
                rz = small.tile([128, 1], F32, tag="rz", name="rz")
                nc.vector.reciprocal_approx_fast(out=rz,
                                                 in_=bank[:, 64:65])
                if on_act:
                    nc.scalar.activation(out=aT[:, 64 * hh:64 * hh + 64],
                                         in_=bank[:, 0:64], func=COPY,
                                         scale=rz)
                else:
                    nc.vector.tensor_scalar_mul(
                        out=aT[:, 64 * hh:64 * hh + 64],
                        in0=bank[:, 0:64], scalar1=rz)

            def emit_transpose(attnp, pairg, nch, on_act=False):
                tp = mm.tile([128, 512], F32, tag="mm",
                             name="tp").bitcast(BF16)
                nc.tensor.transpose(tp[:, 0:128], aT_tiles.pop((pairg, nch)),
                                    ident)
                dst = attnp[pairg % NPAIR][:, 128 * nch:128 * (nch + 1)]
                if on_act:
                    nc.scalar.activation(out=dst, in_=tp[:, 0:128], func=COPY)
                else:
                    nc.vector.tensor_copy(out=dst, in_=tp[:, 0:128])

            def stretch(qt, kt, prevP, curP, fillers=(), front=()):
                """QK+exp for this pair.  Steps 0-7 (par 0) also run the
                PREVIOUS pair's head-1 PV chains + transposes; steps 8-15
                (par 1) run THIS pair's head-0 chains (e[0] is complete by
                then).  Chains therefore trail by only half a stretch and
                never pile up at the boundary."""
                ehd = [[], []]
                nf = len(fillers)
                for i in range(16):
                    par, mt = divmod(i, 8)
                    if qt is not None:
                        s = spool.tile([128, N], F32, tag="s", name="s")
                        for c4 in range(4):
                            nc.tensor.matmul(
                                s[:, 256 * c4:256 * (c4 + 1)],
                                kt[:, par, :, 128 * mt:128 * (mt + 1)],
                                qt[:, par, :, 256 * c4:256 * (c4 + 1)],
                                start=True, stop=True,
                                perf_mode=mybir.MatmulPerfMode.DoubleRow)
                        e = epool.tile([128, N], BF16, tag="e", name="e")
                        nc.scalar.activation(out=e, in_=s, func=EXP,
                                             scale=0.125)
                        ehd[par].append(e)
                    if par == 0 and prevP is not None:
                        vtp, ehdp, attnp, pairgp = prevP
                        emit_chain(vtp, ehdp[1], pairgp, 1, mt)
                        emit_transpose(attnp, pairgp, mt)
                    if par == 1 and curP is not None:
                        vtc, pairgc = curP
                        emit_chain(vtc, ehd[0], pairgc, 0, mt)
                    if i < len(front):
                        front[i]()
                    for j in range(nf * i // 16, nf * (i + 1) // 16):
                        fillers[j]()
                return ehd

            def out_proj_chunk(xf, attn, b, ot, nch):
                ps = mm.tile([128, 512], F32, tag="mm", name="ps")
                for kc in range(KT):
                    nc.tensor.matmul(
                        ps,
                        wo[kc][:, 128 * ot:128 * (ot + 1)],
                        attn[kc][:, 512 * nch:512 * (nch + 1)],
                        start=(kc == 0), stop=(kc == KT - 1))
                osb = osbp.tile([128, 512], F32, tag="osb")
                nc.vector.scalar_tensor_tensor(
                    out=osb, in0=ps, scalar=bo_col[:, ot:ot + 1],
                    in1=xf[ot][:, 512 * nch:512 * (nch + 1)],
                    op0=mybir.AluOpType.add, op1=mybir.AluOpType.add)
                nc.sync.dma_start(
                    out=y[b, 128 * ot:128 * (ot + 1),
                          512 * nch:512 * (nch + 1)],
                    in_=osb)

            # ---- software-pipelined emission ----
            attn_all, xf_all, vT_all = {}, {0: xf0}, {}
            q_all, k_all, e_all = {}, {}, {}
            qf_all, kf_all = {}, {}
            for b in range(NB):
                attn_all[b] = [apool.tile([128, N], BF16, tag=f"attn{t}",
                                          name=f"attn{t}") for t in range(KT)]
                vT_all[b] = None
                for pr in range(NPAIR):
                    q_all[b, pr] = proj_qk_alloc(pr, "q")
                    k_all[b, pr] = proj_qk_alloc(pr, "k")
                    qf_all[b, pr] = fold_alloc(pr, "q")
                    kf_all[b, pr] = fold_alloc(pr, "k")

            def mkjob(fn, *a):
                return lambda: fn(*a)

            def qk_jobs(b, pr):
                xf = xf_all[b]
                jobs = []
                for w in ("q", "k"):
                    src8 = (q_all if w == "q" else k_all)[b, pr]
                    dstf = (qf_all if w == "q" else kf_all)[b, pr]
                    jobs += [mkjob(proj_qk_chunk, xf, pr, w, nch, src8)
                             for nch in range(NCH)]
                    jobs += [mkjob(fold_qk, src8, dstf, hh)
                             for hh in range(2)]
                return jobs

            def vt_jobs(b):
                vT_all[b] = proj_vT_alloc()
                xf = xf_all[b]
                return [mkjob(proj_vT_mtile, xf, vT_all[b][mt], mt)
                        for mt in range(MT)]

            def out_proj_half(xf, attn, b, ot, qh):
                """256-wide out-proj chunk: finer PE granularity so filler
                lumps never delay the next stretch's score matmuls."""
                psf = mm.tile([128, 512], F32, tag="mm", name="psh")
                ps = psf[:, 0:256]
                for kc in range(KT):
                    nc.tensor.matmul(
                        ps,
                        wo[kc][:, 128 * ot:128 * (ot + 1)],
                        attn[kc][:, 256 * qh:256 * (qh + 1)],
                        start=(kc == 0), stop=(kc == KT - 1))
                osb = osbp.tile([128, 256], F32, tag="osbh")
                nc.vector.scalar_tensor_tensor(
                    out=osb, in0=ps, scalar=bo_col[:, ot:ot + 1],
                    in1=xf[ot][:, 256 * qh:256 * (qh + 1)],
                    op0=mybir.AluOpType.add, op1=mybir.AluOpType.add)
                nc.sync.dma_start(
                    out=y[b, 128 * ot:128 * (ot + 1),
                          256 * qh:256 * (qh + 1)],
                    in_=osb)

            def op_jobs(b, halves=False):
                if halves:
                    return [mkjob(out_proj_half, xf_all[b], attn_all[b], b,
                                  ot, qh)
                            for qh in range(4) for ot in range(KT)]
                return [mkjob(out_proj_chunk, xf_all[b], attn_all[b], b,
                              ot, nch)
                        for nch in range(NCH) for ot in range(KT)]

            def prev_of(b, p):
                if (b, p) == (0, 0):
                    return None
                pb, pp = (b, p - 1) if p > 0 else (b - 1, NPAIR - 1)
                return (vT_all[pb], e_all[pb, pp], attn_all[pb],
                        pb * NPAIR + pp)

            # startup: q/k for (0, 0) emitted plainly
            for j in qk_jobs(0, 0):
                j()
            for b in range(NB):
                for pr in range(NPAIR):
                    fillers = []
                    front = []
                    if pr + 1 < NPAIR:
                        fillers += qk_jobs(b, pr + 1)
                    elif b + 1 < NB:
                        fillers += qk_jobs(b + 1, 0)
                    if pr == 0 and b == 0:
                        front = vt_jobs(0)
                    if b + 1 < NB and pr == 1:
                        xf_all[b + 1] = load_xf(b + 1)
                        vt_next = vt_jobs(b + 1)
                        fillers += vt_next[:4]
                    if b + 1 < NB and pr == 2:
                        fillers += vt_next[4:]
                    if b > 0 and pr == 0:
                        fillers += op_jobs(b - 1, halves=True)[:8]
                    if b > 0 and pr == 1:
                        fillers += op_jobs(b - 1, halves=True)[8:]

                    e_all[b, pr] = stretch(qf_all[b, pr], kf_all[b, pr],
                                           prev_of(b, pr),
                                           (vT_all[b], b * NPAIR + pr),
                                           fillers, front)
            # drain: last pair's head-1 chains + transposes interleaved
            # with the out-proj finals (kc3 + SBUF partial + store).
            vtp, ehdp, attnp, pairgp = prev_of(NB, 0)
            ops = op_jobs(NB - 1)
            for nch in range(MT):
                emit_chain(vtp, ehdp[1], pairgp, 1, nch)
                emit_transpose(attnp, pairgp, nch)
                if nch >= 3:
                    ops[nch - 3]()
            for j in ops[5:]:
                j()

    nc.compile()
    return nc


def kernel(x, qkv_w, qkv_b, out_w, out_b, gamma):
    import ml_dtypes
    from concourse.bass_utils import run_bass_kernel_spmd

    x = np.asarray(x, dtype=np.float32)
    qkv_w = np.asarray(qkv_w, dtype=np.float32)
    qkv_b = np.asarray(qkv_b, dtype=np.float32)
    out_w = np.asarray(out_w, dtype=np.float32)
    out_b = np.asarray(out_b, dtype=np.float32)
    gamma = np.asarray(gamma, dtype=np.float32)

    if "nc" not in _cache:
        _cache["nc"] = _build_nc()
    nc = _cache["nc"]

    xf = np.ascontiguousarray(
        x.reshape(B, C, N).astype(ml_dtypes.bfloat16))
    wqkvT = np.ascontiguousarray(
        qkv_w.T.astype(ml_dtypes.bfloat16))                  # (C, 3C) bf16
    bq_col = np.ascontiguousarray(qkv_b.reshape(12, 128).T)  # (128, 12)
    bq_row = np.ascontiguousarray(qkv_b.reshape(1, 3 * C))
    g = gamma.reshape(-1)[0]
    woTg = np.ascontiguousarray(
        (g * out_w).T.astype(ml_dtypes.bfloat16))            # (C, C) bf16
    bog_col = np.ascontiguousarray((g * out_b).reshape(KT, 128).T)
    ident = np.eye(128, dtype=ml_dtypes.bfloat16)

    in_maps = []
    for c in range(NCORES):
        in_maps.append({
            "xs": np.ascontiguousarray(xf[NB * c:NB * (c + 1)]),
            "wqkvT": wqkvT,
            "bqkv_col": bq_col,
            "bqkv_row": bq_row,
            "woTg": woTg,
            "bog_col": bog_col,
            "identd": ident,
        })

    trace = bool(int(os.environ.get("KERNEL_TRACE", "0")))
    try:
        res = run_bass_kernel_spmd(nc, in_maps, core_ids=list(range(NCORES)),
                                   trace=trace)
    except ModuleNotFoundError:
        # NTFF profiling hooks unavailable under this axon client
        res = run_bass_kernel_spmd(nc, in_maps, core_ids=list(range(NCORES)),
                                   trace=False)
    _cache["last_result"] = res

    out = np.concatenate([res.results[c]["y"] for c in range(NCORES)], axis=0)
    return out.reshape(B, C, H, W)
